# revision 36
# baseline (speedup 1.0000x reference)
"""Causal self-attention (GQA + partial RoPE + q_gain) Trainium2 Bass kernel.

Model: B=4, T=2048, D=2048, H=16 q-heads, Hkv=4 kv-heads, hD=128, ROPE=64.
Sharding: 8 cores = 4 batches x 2 head-halves (heads hf*8..hf*8+7, kv heads 2hf, 2hf+1).
Wq/Wkv column-sharded, Wo row-sharded; host sums the two partial outputs per batch.

v10 (~486us, from v2 baseline 660us):
  - host pre-transposes x/Wq/Wk/Wv into partition-major contiguous bf16 blocks
    (v2's strided f32r weight DMAs took 8-20us each and delayed the first
    matmul to t=33us). All matmuls bf16xbf16 (walrus rejects f32r x bf16).
  - steady-state DMA queues: sync = x tiles + output stores; gpsimd = wqh,
    rope swaps, consts, wo. scalar queue only carries wkg/wvg during A0 when
    the ACT engine is idle (engine-issued DMAs block the engine).
  - A(tb+1) projection groups are interleaved into B(tb) as PE filler
    (B0 had 13us of exp-latency bubbles with no C work to hide them); during
    B0 they sit on the pc PSUM banks, which are free until C(0) exists.
    x for block tb+2 prefetched from the middle of B(tb).
  - diagonal score tiles compute only the causally-valid columns
    (scores/exp/pdn/AV restricted to [128r:512]); pt is bf16 so the
    narrow moving operands still run at 1 cycle/row.
  - V projection packs two 256-wide token-chunks into one full PSUM bank
    (second half start=False self-initializes on the pending-zero bytes):
    a half-written bank would leave pending-zero bytes that silently
    discard the DVE-written causal mask of a later diagonal score tile.
  - softmax denominator: non-diagonal exp tiles pre-summed on DVE (bf16 2x)
    in pairs/quads, then one ones-stationary pass per group instead of one
    PE pass per tile (the v2 scheme cost 79us of pure PE overhead).
  - stile triple-buffered so the final C3 drain pipelines.
"""
import numpy as np
import ml_dtypes

import concourse.bass as bass
import concourse.tile as tile
from concourse import bacc, mybir
from concourse.bass_utils import run_bass_kernel_spmd
from contextlib import ExitStack

F32 = mybir.dt.float32
F32R = mybir.dt.float32r
BF16 = mybir.dt.bfloat16
AF = mybir.ActivationFunctionType
AOp = mybir.AluOpType

B, T, D = 4, 2048, 2048
H, Hkv = 16, 4
hD = 128
ROPE = 64
NB = T // 512          # 4 blocks of 512 tokens
HL = H // 2            # 8 heads per core
GL = Hkv // 2          # 2 kv heads per core


def build_nc():
    nc = bacc.Bacc(trn_type="TRN2", target_bir_lowering=False, debug=False)
    xb = nc.dram_tensor("xb", [128, 16, T], BF16, kind="ExternalInput").ap()
    wq = nc.dram_tensor("wq", [128, HL, 16, hD], BF16, kind="ExternalInput").ap()
    wk = nc.dram_tensor("wk", [128, 16, GL * hD], BF16, kind="ExternalInput").ap()
    wv = nc.dram_tensor("wv", [128, 16, GL * hD], BF16, kind="ExternalInput").ap()
    wo = nc.dram_tensor("wo", [HL * hD, D], BF16, kind="ExternalInput").ap()
    cosb = nc.dram_tensor("cosb", [ROPE, T], BF16, kind="ExternalInput").ap()
    sinb = nc.dram_tensor("sinb", [ROPE, T], BF16, kind="ExternalInput").ap()
    maskb = nc.dram_tensor("maskb", [128, 4, 512], BF16, kind="ExternalInput").ap()
    onesb = nc.dram_tensor("onesb", [128, 128], BF16, kind="ExternalInput").ap()
    outT = nc.dram_tensor("outT", [D, T], F32, kind="ExternalOutput").ap()

    with tile.TileContext(nc) as tc, ExitStack() as ctx:
        const = ctx.enter_context(tc.tile_pool(name="const", bufs=1))
        wpool = ctx.enter_context(tc.tile_pool(name="wpool", bufs=1))
        persist = ctx.enter_context(tc.tile_pool(name="persist", bufs=1))
        xpool = ctx.enter_context(tc.tile_pool(name="xp", bufs=48))
        qbp = ctx.enter_context(tc.tile_pool(name="qbp", bufs=18))
        swpool = ctx.enter_context(tc.tile_pool(name="swp", bufs=2))
        wqp = ctx.enter_context(tc.tile_pool(name="wqp", bufs=3))
        rtmp = ctx.enter_context(tc.tile_pool(name="rtmp", bufs=1))
        ptp = ctx.enter_context(tc.tile_pool(name="ptp", bufs=8))
        pt2p = ctx.enter_context(tc.tile_pool(name="pt2p", bufs=4))
        rpsp = ctx.enter_context(tc.tile_pool(name="rpsp", bufs=1))
        otp = ctx.enter_context(tc.tile_pool(name="otp", bufs=2))
        stilep = ctx.enter_context(tc.tile_pool(name="stilep", bufs=3))
        ps = ctx.enter_context(tc.tile_pool(name="ps", bufs=1, space="PSUM"))

        # rope tables first on gpsimd (needed by A0's K rope); tones/tmask
        # deferred (first needed in B0) so A0's wqh loads go out earlier
        tcos = const.tile([ROPE, T], BF16, tag="tcos")
        nc.gpsimd.dma_start(tcos[:], cosb)
        tsin = const.tile([ROPE, T], BF16, tag="tsin")
        nc.gpsimd.dma_start(tsin[:], sinb)
        tones = const.tile([128, 128], BF16, tag="tones")
        tmask = const.tile([128, 4, 512], BF16, tag="tmask")

        # K/V weights on the scalar queue: ACT is idle during A0.
        # wkg in 4 chunks so the first K matmul starts after ~1/4 transfer.
        wkg = wpool.tile([128, 16, GL * hD], BF16, tag="wkg")
        for q in range(4):
            nc.scalar.dma_start(wkg[:, 4 * q:4 * (q + 1), :], wk[:, 4 * q:4 * (q + 1), :])
        wvg = wpool.tile([128, 16, GL * hD], BF16, tag="wvg")
        wo_t = []
        for j in range(HL):
            w = wpool.tile([128, D], BF16, tag=f"wo{j}", name=f"wo{j}")
            wo_t.append(w)

        KT = persist.tile([128, GL, T], BF16, tag="KT")
        VT = persist.tile([128, T // 128, GL * hD], BF16, tag="VT")

        qbs = {tb: [] for tb in range(NB)}   # tb -> 8 qb tiles
        xts = {}                             # tb -> 16 x tiles
        OTh = {}                             # tb -> OT tile [128, HL, 512] bf16
        c_pend = []                          # queued phase-C groups: (tb, m2)

        def issue_x(tb):
            tsl = slice(512 * tb, 512 * (tb + 1))
            lst = []
            for d in range(16):
                xt = xpool.tile([128, 512], BF16, tag="xt", name=f"xt{tb}_{d}")
                nc.sync.dma_start(xt[:], xb[:, d, tsl])
                lst.append(xt)
            xts[tb] = lst

        def xap(tb, d):
            return xts[tb][d][:]

        def emit_c_group(ptag="pc", pbufs=2):
            tbc, m2 = c_pend.pop(0)
            msl = slice(128 * m2, 128 * (m2 + 1))
            csl = slice(512 * tbc, 512 * (tbc + 1))
            pc = ps.tile([128, 512], F32, tag=ptag, bufs=pbufs, name=f"pc{tbc}_{m2}")
            for j in range(HL):
                nc.tensor.matmul(pc[:], wo_t[j][:, msl], OTh[tbc][:][:, j, :],
                                 start=(j == 0), stop=(j == HL - 1))
            stile = stilep.tile([128, 512], F32, tag="stile", name=f"st{tbc}_{m2}")
            nc.vector.tensor_copy(stile[:], pc[:])
            nc.sync.dma_start(outT[msl, csl], stile[:])

        def rope(dst, src_ap, tsl, nm):
            # dst[0:64, :] = src[0:64]*C + swap(src)*S (C/S hold the +-sin trick)
            sw = swpool.tile([ROPE, 512], BF16, tag="sw", name=f"sw{nm}")
            nc.gpsimd.dma_start(sw[0:32, :], src_ap[32:64])
            nc.gpsimd.dma_start(sw[32:64, :], src_ap[0:32])
            ts_ = rtmp.tile([ROPE, 512], F32R, tag="ts_", name=f"ts{nm}")
            tc_ = rtmp.tile([ROPE, 512], F32R, tag="tc_", name=f"tc{nm}")
            nc.vector.tensor_mul(ts_[:], sw[:], tsin[:, tsl])
            nc.vector.tensor_mul(tc_[:], src_ap[0:ROPE], tcos[:, tsl])
            nc.vector.tensor_tensor(out=dst[0:ROPE], in0=tc_[:], in1=ts_[:], op=AOp.add)

        def make_a_groups(tb, ptag="mm", pbufs=3):
            """Closures for A(tb)'s matmul groups: 2 K, 2 V-pair, 8 Q."""
            tsl = slice(512 * tb, 512 * (tb + 1))
            groups = []
            wqhs = {}

            def load_wqh(h):
                # sync queue: gpsimd is in-order behind rope-swap DMAs that
                # themselves wait on DVE, which stalled Q groups on wqh
                if h < HL and h not in wqhs:
                    wqh = wqp.tile([128, 16, hD], BF16, tag="wqh", name=f"wq{tb}_{h}")
                    nc.sync.dma_start(wqh[:], wq[:, h])
                    wqhs[h] = wqh

            def kg(g):
                pk = ps.tile([128, 512], F32, tag=ptag, bufs=pbufs, name=f"pk{tb}_{g}")
                for d in range(16):
                    nc.tensor.matmul(pk[:], wkg[:][:, d, 128 * g:128 * (g + 1)],
                                     xap(tb, d), start=(d == 0), stop=(d == 15))
                nc.vector.tensor_copy(KT[:][:, g, tsl], pk[:])
                rope(KT[:][:, g, tsl], KT[:][:, g, tsl], tsl, f"k{tb}_{g}")

            def vg(tp):
                # two token-chunks packed into one full PSUM bank: the second
                # half's d=0 matmul uses start=False and lands on the
                # pending-zero bytes left by the first half's start=True, so
                # it self-initializes. Leaving the bank half-written would
                # poison later diagonal score tiles (mask DVE-write discarded
                # on pending-zero bytes).
                pv = ps.tile([128, 512], F32, tag=ptag, bufs=pbufs, name=f"pv{tb}_{tp}")
                for half in range(2):
                    tloc = 2 * tp + half
                    for d in range(16):
                        nc.tensor.matmul(pv[:, 256 * half:256 * (half + 1)],
                                         xts[tb][d][:, 128 * tloc:128 * (tloc + 1)],
                                         wvg[:][:, d, :],
                                         start=(d == 0 and half == 0),
                                         stop=(d == 15 and half == 1),
                                         skip_group_check=(half == 1))
                nc.vector.tensor_copy(
                    VT[:][:, 4 * tb + 2 * tp:4 * tb + 2 * tp + 2, :], pv[:])

            def qg(h):
                load_wqh(h)          # no-op if already prefetched
                load_wqh(h + 1)      # prefetch next heads' weights
                load_wqh(h + 2)
                wqh = wqhs.pop(h)
                pq = ps.tile([128, 512], F32, tag=ptag, bufs=pbufs, name=f"pq{tb}_{h}")
                for d in range(16):
                    nc.tensor.matmul(pq[:], wqh[:][:, d, :],
                                     xap(tb, d), start=(d == 0), stop=(d == 15))
                qb = qbp.tile([128, 512], BF16, tag="qb", name=f"qb{tb}_{h}")
                nc.vector.tensor_copy(qb[:], pq[:])
                rope(qb[:], qb[:], tsl, f"q{tb}_{h}")
                qbs[tb].append(qb)

            load_wqh(0)
            load_wqh(1)
            for g in range(GL):
                groups.append(lambda g=g: kg(g))
            for tp in range(2):
                groups.append(lambda tp=tp: vg(tp))
            for h in range(HL):
                groups.append(lambda h=h: qg(h))
            return groups

        def emit_b_head(tb, h, mid_filler=None):
            nj = 4 * tb + 4
            ndiag = 4 * tb          # j < ndiag are full-width (non-diagonal)
            g = h // (HL // GL)
            po = ps.tile([128, 512], F32, tag="po", bufs=2, name=f"po{tb}_{h}")
            pdn = ps.tile([128, 512], F32, tag="pdn", bufs=1, name=f"pd{tb}_{h}")
            pts = []
            slices = []
            pairs = []
            started = [False]

            def pdn_mm(src_ap, sl, last):
                st_, started[0] = not started[0], True
                nc.tensor.matmul(pdn[:, sl], tones[:], src_ap,
                                 start=st_, stop=last,
                                 skip_group_check=not st_)

            def emit_av(j):
                sl = slices[j]
                part = sl != slice(0, 512)
                if j >= ndiag:
                    # diagonal: per-j denominator on the valid columns
                    pdn_mm(pts[j][:][:, sl], sl, j == nj - 1)
                elif j & 1:
                    # non-diagonal pair (j-1, j): one DVE bf16 add, then a
                    # single denominator pass per pair (or per quad for the
                    # deeper blocks, where DVE still has slack)
                    p2 = pt2p.tile([128, 512], BF16, tag="pt2",
                                   name=f"p2{tb}_{h}_{j}")
                    nc.vector.tensor_tensor(out=p2[:], in0=pts[j - 1][:],
                                            in1=pts[j][:], op=AOp.add)
                    if ndiag >= 8 and (j & 3) == 1 and j + 2 < ndiag:
                        pairs.append(p2)     # defer: quad partner comes at j+2
                    elif ndiag >= 8 and (j & 3) == 3 and pairs:
                        p4 = pt2p.tile([128, 512], BF16, tag="pt2",
                                       name=f"p4{tb}_{h}_{j}")
                        nc.vector.tensor_tensor(out=p4[:], in0=pairs.pop()[:],
                                                in1=p2[:], op=AOp.add)
                        pdn_mm(p4[:], slice(0, 512), False)
                    else:
                        pdn_mm(p2[:], slice(0, 512), False)
                nc.tensor.matmul(po[:, sl], VT[:][:, j, 128 * g:128 * (g + 1)],
                                 pts[j][:][:, sl], start=(j == 0), stop=(j == nj - 1),
                                 skip_group_check=part)

            for j in range(nj):
                if j == nj - 1 and mid_filler is not None:
                    # PE filler just before the exp-latency-exposed tail
                    mid_filler()
                r = j - 4 * tb
                st = ps.tile([128, 512], F32, tag="mm", bufs=3,
                             name=f"s{tb}_{h}_{j}")
                if r >= 0:
                    sl = slice(128 * r, 512)
                    # mask-init on ACT (same table set as Exp, no reload):
                    # DVE is the congested engine during B phases
                    nc.scalar.activation(st[:, sl], tmask[:][:, r, sl], AF.Copy)
                    nc.tensor.matmul(st[:, sl], KT[:][:, g, 128 * j:128 * (j + 1)],
                                     qbs[tb][h][:][:, sl], start=False, stop=True,
                                     skip_group_check=True)
                else:
                    sl = slice(0, 512)
                    nc.tensor.matmul(st[:], KT[:][:, g, 128 * j:128 * (j + 1)],
                                     qbs[tb][h][:], start=True, stop=True)
                pt = ptp.tile([128, 512], BF16, tag="pt", name=f"p{tb}_{h}_{j}")
                nc.scalar.activation(pt[:, sl], st[:, sl], AF.Exp)
                pts.append(pt)
                slices.append(sl)
                if j >= 3:
                    emit_av(j - 3)
            for j in range(max(nj - 3, 0), nj):
                emit_av(j)
            rps = rpsp.tile([128, 512], F32, tag="rps", name=f"r{tb}_{h}")
            nc.vector.reciprocal_approx_fast(rps[:], pdn[:])
            nc.vector.tensor_tensor(out=OTh[tb][:][:, h, :], in0=po[:],
                                    in1=rps[:], op=AOp.mult)

        # ---------------- A0 standalone ----------------
        with nc.named_scope("A0"):
            issue_x(0)
            nc.scalar.dma_start(wvg[:], wv)
            for grp in make_a_groups(0):
                grp()
            issue_x(1)   # A1 filler groups inside B0 need these early
            # deferred consts (first needed in B0), then Wo (needed in B1)
            nc.gpsimd.dma_start(tones[:], onesb)
            nc.gpsimd.dma_start(tmask[:], maskb)
            for j in range(HL):
                nc.gpsimd.dma_start(wo_t[j][:], wo[128 * j:128 * (j + 1), :])

        # ---------------- B(tb) with A(tb+1) + C(tb-1) filler ----------------
        for tb in range(NB):
            with nc.named_scope(f"B{tb}"):
                OTh[tb] = otp.tile([128, HL, 512], BF16, tag="OT", name=f"OT{tb}")
                filler = []
                if tb + 1 < NB:
                    # during B0 the pc banks are free (no C work queued yet):
                    # keep A1 filler groups off the score-tile rotation
                    filler = (make_a_groups(tb + 1, ptag="pc", pbufs=2)
                              if tb == 0 else make_a_groups(tb + 1))
                fstate = [0]

                def pop_filler():
                    if fstate[0] < len(filler):
                        filler[fstate[0]]()
                        fstate[0] += 1

                for h in range(HL):
                    if h == 3 and tb + 2 < NB:
                        issue_x(tb + 2)   # prefetch next block's x mid-B
                    # B0 heads are all-diagonal and short: place one filler
                    # group right before the exp-latency-exposed tail
                    emit_b_head(tb, h, mid_filler=pop_filler if tb == 0 else None)
                    for _ in range(2):
                        if c_pend:
                            emit_c_group()
                    # B0: 12 filler groups over 8 heads = 1 mid each + post
                    # only for the first 4 heads, so the tail heads keep
                    # their mid-filler coverage
                    for _ in range((1 if h < 4 else 0) if tb == 0 else 2):
                        pop_filler()
                while fstate[0] < len(filler):
                    pop_filler()
            c_pend.extend((tb, m2) for m2 in range(16))

        # drain the last block's output projection; B work is done, so
        # rotate over the mm banks too (5-deep pipelining instead of 2)
        with nc.named_scope("C3"):
            alt = 0
            while c_pend:
                if alt % 5 < 3:
                    emit_c_group(ptag="mm", pbufs=3)
                else:
                    emit_c_group()
                alt += 1
    nc.compile()
    return nc


# de-interleave permutation for rope channels: x1 (even) -> 0:32, x2 (odd) -> 32:64
_PERM = np.concatenate([np.arange(0, ROPE, 2), np.arange(1, ROPE, 2), np.arange(ROPE, hD)])


def prepare_inputs(x, cos, sin, Wq, Wkv, Wo, q_gain):
    """Host-side sharding + layout prep. Returns list of 8 in_maps."""
    x = np.asarray(x, np.float32)
    cos = np.asarray(cos, np.float32)
    sin = np.asarray(sin, np.float32)
    Wq = np.asarray(Wq, np.float32)
    Wkv = np.asarray(Wkv, np.float32)
    Wo = np.asarray(Wo, np.float32)
    q_gain = np.asarray(q_gain, np.float32)
    bf = ml_dtypes.bfloat16

    # rope tables in de-interleaved order: C = [cos; cos], S = [-sin; +sin]
    cosb = np.ascontiguousarray(np.concatenate([cos.T, cos.T], axis=0)).astype(bf)
    sinb = np.ascontiguousarray(np.concatenate([-sin.T, sin.T], axis=0)).astype(bf)

    # additive causal masks for diagonal s-tiles, r = j - 4*ib
    p = np.arange(128)[:, None]
    f = np.arange(512)[None, :]
    maskb = np.zeros((128, 4, 512), np.float32)
    for r in range(4):
        maskb[:, r, :] = np.where(p + 128 * r > f, -1e9, 0.0)
    maskb = maskb.astype(bf)

    onesb = np.ones((128, 128), np.float32)
    scale = 1.0 / np.sqrt(hD)

    # x: per batch [128, 16, T] partition-major contiguous bf16
    xb = [np.ascontiguousarray(
        x[b].T.reshape(16, 128, T).transpose(1, 0, 2)).astype(bf) for b in range(B)]

    in_maps = []
    for c in range(8):
        b, hf = divmod(c, 2)
        heads = np.arange(hf * HL, (hf + 1) * HL)
        Wq_h = Wq.reshape(H, hD, D)[heads] * (q_gain[heads, None, None] * scale)
        Wq_h = Wq_h[:, _PERM, :]                                     # de-interleave rope chans
        kvh = np.arange(hf * GL, (hf + 1) * GL)
        Wkv_r = Wkv.reshape(Hkv, 2 * hD, D)[kvh]
        Wk_h = Wkv_r[:, :hD, :][:, _PERM, :]
        Wv_h = Wkv_r[:, hD:, :]
        Wo_h = Wo[:, hf * HL * hD:(hf + 1) * HL * hD]

        # [D, M] transposed weights -> [128, 16, M] partition-major contiguous
        WqT = Wq_h.reshape(HL * hD, D).T        # [D, 1024]
        wq_host = np.ascontiguousarray(
            WqT.reshape(16, 128, HL, hD).transpose(1, 2, 0, 3)).astype(bf)
        WkT = Wk_h.reshape(GL * hD, D).T        # [D, 256]
        wk_host = np.ascontiguousarray(
            WkT.reshape(16, 128, GL * hD).transpose(1, 0, 2)).astype(bf)
        WvT = Wv_h.reshape(GL * hD, D).T
        wv_host = np.ascontiguousarray(
            WvT.reshape(16, 128, GL * hD).transpose(1, 0, 2)).astype(bf)

        in_maps.append({
            "xb": xb[b],
            "wq": wq_host,
            "wk": wk_host,
            "wv": wv_host,
            "wo": np.ascontiguousarray(Wo_h.T).astype(bf),
            "cosb": cosb, "sinb": sinb, "maskb": maskb,
            "onesb": onesb.astype(bf),
        })
    return in_maps


_NC_CACHE = {}


def kernel(x, cos, sin, Wq, Wkv, Wo, q_gain, _trace=False):
    if "nc" not in _NC_CACHE:
        _NC_CACHE["nc"] = build_nc()
    nc = _NC_CACHE["nc"]
    in_maps = prepare_inputs(x, cos, sin, Wq, Wkv, Wo, q_gain)
    res = run_bass_kernel_spmd(nc, in_maps, core_ids=list(range(8)), trace=_trace)
    if _trace:
        _NC_CACHE["last_results"] = res
    out = np.empty((B, T, D), np.float32)
    for b in range(B):
        acc = res.results[2 * b]["outT"] + res.results[2 * b + 1]["outT"]
        out[b] = acc.T
    return out


# revision 40
# speedup vs baseline: 1.0042x; 1.0042x over previous
"""Causal self-attention (GQA + partial RoPE + q_gain) Trainium2 Bass kernel.

Model: B=4, T=2048, D=2048, H=16 q-heads, Hkv=4 kv-heads, hD=128, ROPE=64.
Sharding: 8 cores = 4 batches x 2 head-halves (heads hf*8..hf*8+7, kv heads 2hf, 2hf+1).
Wq/Wkv column-sharded, Wo row-sharded; host sums the two partial outputs per batch.

v13 (~473us, from v2 baseline 660us):
  - diagonal-mask inits run on ACT (Copy shares Exp's table set, no reload);
    DVE was the congested engine during B phases.
  - wqh loads on the sync queue, two heads ahead (gpsimd is in-order behind
    rope-swap DMAs that wait on DVE).
  - final C3 drain rotates over the mm+pc PSUM banks (5-deep).
Earlier stages:
  - host pre-transposes x/Wq/Wk/Wv into partition-major contiguous bf16 blocks
    (v2's strided f32r weight DMAs took 8-20us each and delayed the first
    matmul to t=33us). All matmuls bf16xbf16 (walrus rejects f32r x bf16).
  - steady-state DMA queues: sync = x tiles + output stores; gpsimd = wqh,
    rope swaps, consts, wo. scalar queue only carries wkg/wvg during A0 when
    the ACT engine is idle (engine-issued DMAs block the engine).
  - A(tb+1) projection groups are interleaved into B(tb) as PE filler
    (B0 had 13us of exp-latency bubbles with no C work to hide them); during
    B0 they sit on the pc PSUM banks, which are free until C(0) exists.
    x for block tb+2 prefetched from the middle of B(tb).
  - diagonal score tiles compute only the causally-valid columns
    (scores/exp/pdn/AV restricted to [128r:512]); pt is bf16 so the
    narrow moving operands still run at 1 cycle/row.
  - V projection packs two 256-wide token-chunks into one full PSUM bank
    (second half start=False self-initializes on the pending-zero bytes):
    a half-written bank would leave pending-zero bytes that silently
    discard the DVE-written causal mask of a later diagonal score tile.
  - softmax denominator: non-diagonal exp tiles pre-summed on DVE (bf16 2x)
    in pairs/quads, then one ones-stationary pass per group instead of one
    PE pass per tile (the v2 scheme cost 79us of pure PE overhead).
  - stile triple-buffered so the final C3 drain pipelines.
"""
import numpy as np
import ml_dtypes

import concourse.bass as bass
import concourse.tile as tile
from concourse import bacc, mybir
from concourse.bass_utils import run_bass_kernel_spmd
from contextlib import ExitStack

F32 = mybir.dt.float32
F32R = mybir.dt.float32r
BF16 = mybir.dt.bfloat16
AF = mybir.ActivationFunctionType
AOp = mybir.AluOpType

B, T, D = 4, 2048, 2048
H, Hkv = 16, 4
hD = 128
ROPE = 64
NB = T // 512          # 4 blocks of 512 tokens
HL = H // 2            # 8 heads per core
GL = Hkv // 2          # 2 kv heads per core


def build_nc():
    nc = bacc.Bacc(trn_type="TRN2", target_bir_lowering=False, debug=False)
    xb = nc.dram_tensor("xb", [128, 16, T], BF16, kind="ExternalInput").ap()
    wq = nc.dram_tensor("wq", [128, HL, 16, hD], BF16, kind="ExternalInput").ap()
    wk = nc.dram_tensor("wk", [128, 16, GL * hD], BF16, kind="ExternalInput").ap()
    wv = nc.dram_tensor("wv", [128, 16, GL * hD], BF16, kind="ExternalInput").ap()
    wo = nc.dram_tensor("wo", [HL * hD, D], BF16, kind="ExternalInput").ap()
    cosb = nc.dram_tensor("cosb", [ROPE, T], BF16, kind="ExternalInput").ap()
    sinb = nc.dram_tensor("sinb", [ROPE, T], BF16, kind="ExternalInput").ap()
    maskb = nc.dram_tensor("maskb", [128, 4, 512], BF16, kind="ExternalInput").ap()
    onesb = nc.dram_tensor("onesb", [128, 128], BF16, kind="ExternalInput").ap()
    outT = nc.dram_tensor("outT", [D, T], F32, kind="ExternalOutput").ap()

    with tile.TileContext(nc) as tc, ExitStack() as ctx:
        const = ctx.enter_context(tc.tile_pool(name="const", bufs=1))
        wpool = ctx.enter_context(tc.tile_pool(name="wpool", bufs=1))
        persist = ctx.enter_context(tc.tile_pool(name="persist", bufs=1))
        xpool = ctx.enter_context(tc.tile_pool(name="xp", bufs=48))
        qbp = ctx.enter_context(tc.tile_pool(name="qbp", bufs=18))
        swpool = ctx.enter_context(tc.tile_pool(name="swp", bufs=2))
        wqp = ctx.enter_context(tc.tile_pool(name="wqp", bufs=3))
        rtmp = ctx.enter_context(tc.tile_pool(name="rtmp", bufs=1))
        ptp = ctx.enter_context(tc.tile_pool(name="ptp", bufs=8))
        pt2p = ctx.enter_context(tc.tile_pool(name="pt2p", bufs=4))
        rpsp = ctx.enter_context(tc.tile_pool(name="rpsp", bufs=1))
        otp = ctx.enter_context(tc.tile_pool(name="otp", bufs=2))
        stilep = ctx.enter_context(tc.tile_pool(name="stilep", bufs=3))
        ps = ctx.enter_context(tc.tile_pool(name="ps", bufs=1, space="PSUM"))

        # rope tables first on gpsimd (needed by A0's K rope); tones/tmask
        # deferred (first needed in B0) so A0's wqh loads go out earlier
        tcos = const.tile([ROPE, T], BF16, tag="tcos")
        nc.gpsimd.dma_start(tcos[:], cosb)
        tsin = const.tile([ROPE, T], BF16, tag="tsin")
        nc.gpsimd.dma_start(tsin[:], sinb)
        tones = const.tile([128, 128], BF16, tag="tones")
        tmask = const.tile([128, 4, 512], BF16, tag="tmask")

        # K/V weights on the scalar queue: ACT is idle during A0.
        # wkg in 4 chunks so the first K matmul starts after ~1/4 transfer.
        wkg = wpool.tile([128, 16, GL * hD], BF16, tag="wkg")
        for q in range(4):
            nc.scalar.dma_start(wkg[:, 4 * q:4 * (q + 1), :], wk[:, 4 * q:4 * (q + 1), :])
        wvg = wpool.tile([128, 16, GL * hD], BF16, tag="wvg")
        wo_t = []
        for j in range(HL):
            w = wpool.tile([128, D], BF16, tag=f"wo{j}", name=f"wo{j}")
            wo_t.append(w)

        KT = persist.tile([128, GL, T], BF16, tag="KT")
        VT = persist.tile([128, T // 128, GL * hD], BF16, tag="VT")

        qbs = {tb: [] for tb in range(NB)}   # tb -> 8 qb tiles
        xts = {}                             # tb -> 16 x tiles
        OTh = {}                             # tb -> OT tile [128, HL, 512] bf16
        c_pend = []                          # queued phase-C groups: (tb, m2)

        def issue_x(tb):
            tsl = slice(512 * tb, 512 * (tb + 1))
            lst = []
            for d in range(16):
                xt = xpool.tile([128, 512], BF16, tag="xt", name=f"xt{tb}_{d}")
                nc.sync.dma_start(xt[:], xb[:, d, tsl])
                lst.append(xt)
            xts[tb] = lst

        def xap(tb, d):
            return xts[tb][d][:]

        def emit_c_group(ptag="pc", pbufs=2):
            tbc, m2 = c_pend.pop(0)
            msl = slice(128 * m2, 128 * (m2 + 1))
            csl = slice(512 * tbc, 512 * (tbc + 1))
            pc = ps.tile([128, 512], F32, tag=ptag, bufs=pbufs, name=f"pc{tbc}_{m2}")
            for j in range(HL):
                nc.tensor.matmul(pc[:], wo_t[j][:, msl], OTh[tbc][:][:, j, :],
                                 start=(j == 0), stop=(j == HL - 1))
            stile = stilep.tile([128, 512], F32, tag="stile", name=f"st{tbc}_{m2}")
            nc.vector.tensor_copy(stile[:], pc[:])
            nc.sync.dma_start(outT[msl, csl], stile[:])

        def rope(dst, src_ap, tsl, nm):
            # dst[0:64, :] = src[0:64]*C + swap(src)*S (C/S hold the +-sin trick)
            sw = swpool.tile([ROPE, 512], BF16, tag="sw", name=f"sw{nm}")
            nc.gpsimd.dma_start(sw[0:32, :], src_ap[32:64])
            nc.gpsimd.dma_start(sw[32:64, :], src_ap[0:32])
            ts_ = rtmp.tile([ROPE, 512], F32R, tag="ts_", name=f"ts{nm}")
            tc_ = rtmp.tile([ROPE, 512], F32R, tag="tc_", name=f"tc{nm}")
            nc.vector.tensor_mul(ts_[:], sw[:], tsin[:, tsl])
            nc.vector.tensor_mul(tc_[:], src_ap[0:ROPE], tcos[:, tsl])
            nc.vector.tensor_tensor(out=dst[0:ROPE], in0=tc_[:], in1=ts_[:], op=AOp.add)

        def make_a_groups(tb, ptag="mm", pbufs=3):
            """Closures for A(tb)'s matmul groups: 2 K, 2 V-pair, 8 Q."""
            tsl = slice(512 * tb, 512 * (tb + 1))
            groups = []
            wqhs = {}

            def load_wqh(h):
                # sync queue: gpsimd is in-order behind rope-swap DMAs that
                # themselves wait on DVE, which stalled Q groups on wqh
                if h < HL and h not in wqhs:
                    wqh = wqp.tile([128, 16, hD], BF16, tag="wqh", name=f"wq{tb}_{h}")
                    nc.sync.dma_start(wqh[:], wq[:, h])
                    wqhs[h] = wqh

            def kg(g):
                pk = ps.tile([128, 512], F32, tag=ptag, bufs=pbufs, name=f"pk{tb}_{g}")
                for d in range(16):
                    nc.tensor.matmul(pk[:], wkg[:][:, d, 128 * g:128 * (g + 1)],
                                     xap(tb, d), start=(d == 0), stop=(d == 15))
                nc.vector.tensor_copy(KT[:][:, g, tsl], pk[:])
                rope(KT[:][:, g, tsl], KT[:][:, g, tsl], tsl, f"k{tb}_{g}")

            def vg(tp):
                # two token-chunks packed into one full PSUM bank: the second
                # half's d=0 matmul uses start=False and lands on the
                # pending-zero bytes left by the first half's start=True, so
                # it self-initializes. Leaving the bank half-written would
                # poison later diagonal score tiles (mask DVE-write discarded
                # on pending-zero bytes).
                pv = ps.tile([128, 512], F32, tag=ptag, bufs=pbufs, name=f"pv{tb}_{tp}")
                for half in range(2):
                    tloc = 2 * tp + half
                    for d in range(16):
                        nc.tensor.matmul(pv[:, 256 * half:256 * (half + 1)],
                                         xts[tb][d][:, 128 * tloc:128 * (tloc + 1)],
                                         wvg[:][:, d, :],
                                         start=(d == 0 and half == 0),
                                         stop=(d == 15 and half == 1),
                                         skip_group_check=(half == 1))
                nc.vector.tensor_copy(
                    VT[:][:, 4 * tb + 2 * tp:4 * tb + 2 * tp + 2, :], pv[:])

            def qg(h):
                load_wqh(h)          # no-op if already prefetched
                load_wqh(h + 1)      # prefetch next heads' weights
                load_wqh(h + 2)
                wqh = wqhs.pop(h)
                pq = ps.tile([128, 512], F32, tag=ptag, bufs=pbufs, name=f"pq{tb}_{h}")
                for d in range(16):
                    nc.tensor.matmul(pq[:], wqh[:][:, d, :],
                                     xap(tb, d), start=(d == 0), stop=(d == 15))
                qb = qbp.tile([128, 512], BF16, tag="qb", name=f"qb{tb}_{h}")
                nc.vector.tensor_copy(qb[:], pq[:])
                rope(qb[:], qb[:], tsl, f"q{tb}_{h}")
                qbs[tb].append(qb)

            load_wqh(0)
            load_wqh(1)
            for g in range(GL):
                groups.append(lambda g=g: kg(g))
            for tp in range(2):
                groups.append(lambda tp=tp: vg(tp))
            for h in range(HL):
                groups.append(lambda h=h: qg(h))
            return groups

        def emit_b_head(tb, h, mid_filler=None):
            nj = 4 * tb + 4
            ndiag = 4 * tb          # j < ndiag are full-width (non-diagonal)
            g = h // (HL // GL)
            po = ps.tile([128, 512], F32, tag="po", bufs=2, name=f"po{tb}_{h}")
            pdn = ps.tile([128, 512], F32, tag="pdn", bufs=1, name=f"pd{tb}_{h}")
            pts = []
            slices = []
            pairs = []
            started = [False]

            def pdn_mm(src_ap, sl, last):
                st_, started[0] = not started[0], True
                nc.tensor.matmul(pdn[:, sl], tones[:], src_ap,
                                 start=st_, stop=last,
                                 skip_group_check=not st_)

            def emit_av(j):
                sl = slices[j]
                part = sl != slice(0, 512)
                if j >= ndiag:
                    # diagonal: per-j denominator on the valid columns
                    pdn_mm(pts[j][:][:, sl], sl, j == nj - 1)
                elif j & 1:
                    # non-diagonal pair (j-1, j) folded into a running DVE
                    # bf16 sum; ONE denominator pass per head for all
                    # non-diagonal tiles (values <= ndiag, fine in bf16)
                    p2 = pt2p.tile([128, 512], BF16, tag="pt2",
                                   name=f"p2{tb}_{h}_{j}")
                    nc.vector.tensor_tensor(out=p2[:], in0=pts[j - 1][:],
                                            in1=pts[j][:], op=AOp.add)
                    if pairs:
                        run = pt2p.tile([128, 512], BF16, tag="pt2",
                                        name=f"run{tb}_{h}_{j}")
                        nc.vector.tensor_tensor(out=run[:], in0=pairs.pop()[:],
                                                in1=p2[:], op=AOp.add)
                    else:
                        run = p2
                    if j == ndiag - 1:
                        pdn_mm(run[:], slice(0, 512), False)
                    else:
                        pairs.append(run)
                nc.tensor.matmul(po[:, sl], VT[:][:, j, 128 * g:128 * (g + 1)],
                                 pts[j][:][:, sl], start=(j == 0), stop=(j == nj - 1),
                                 skip_group_check=part)

            for j in range(nj):
                if j == nj - 1 and mid_filler is not None:
                    # PE filler just before the exp-latency-exposed tail
                    mid_filler()
                r = j - 4 * tb
                st = ps.tile([128, 512], F32, tag="mm", bufs=3,
                             name=f"s{tb}_{h}_{j}")
                if r >= 0:
                    sl = slice(128 * r, 512)
                    if tb == 0:
                        # B0 is all-diagonal: ACT would carry exps AND masks
                        # and become the critical stream; DVE has slack there
                        nc.vector.tensor_copy(st[:, sl], tmask[:][:, r, sl])
                    else:
                        # mask-init on ACT (same table set as Exp, no
                        # reload): DVE is the congested engine in B1-B3
                        nc.scalar.activation(st[:, sl], tmask[:][:, r, sl],
                                             AF.Copy)
                    nc.tensor.matmul(st[:, sl], KT[:][:, g, 128 * j:128 * (j + 1)],
                                     qbs[tb][h][:][:, sl], start=False, stop=True,
                                     skip_group_check=True)
                else:
                    sl = slice(0, 512)
                    nc.tensor.matmul(st[:], KT[:][:, g, 128 * j:128 * (j + 1)],
                                     qbs[tb][h][:], start=True, stop=True)
                pt = ptp.tile([128, 512], BF16, tag="pt", name=f"p{tb}_{h}_{j}")
                nc.scalar.activation(pt[:, sl], st[:, sl], AF.Exp)
                pts.append(pt)
                slices.append(sl)
                if j >= 3:
                    emit_av(j - 3)
            for j in range(max(nj - 3, 0), nj):
                emit_av(j)
            rps = rpsp.tile([128, 512], F32, tag="rps", name=f"r{tb}_{h}")
            nc.vector.reciprocal_approx_fast(rps[:], pdn[:])
            nc.vector.tensor_tensor(out=OTh[tb][:][:, h, :], in0=po[:],
                                    in1=rps[:], op=AOp.mult)

        # ---------------- A0 standalone ----------------
        with nc.named_scope("A0"):
            issue_x(0)
            nc.scalar.dma_start(wvg[:], wv)
            for grp in make_a_groups(0):
                grp()
            issue_x(1)   # A1 filler groups inside B0 need these early
            # deferred consts (first needed in B0), then Wo (needed in B1)
            nc.gpsimd.dma_start(tones[:], onesb)
            nc.gpsimd.dma_start(tmask[:], maskb)
            for j in range(HL):
                nc.gpsimd.dma_start(wo_t[j][:], wo[128 * j:128 * (j + 1), :])

        # ---------------- B(tb) with A(tb+1) + C(tb-1) filler ----------------
        for tb in range(NB):
            with nc.named_scope(f"B{tb}"):
                OTh[tb] = otp.tile([128, HL, 512], BF16, tag="OT", name=f"OT{tb}")
                filler = []
                if tb + 1 < NB:
                    # during B0 the pc banks are free (no C work queued yet):
                    # keep A1 filler groups off the score-tile rotation
                    filler = (make_a_groups(tb + 1, ptag="pc", pbufs=2)
                              if tb == 0 else make_a_groups(tb + 1))
                fstate = [0]

                def pop_filler():
                    if fstate[0] < len(filler):
                        filler[fstate[0]]()
                        fstate[0] += 1

                for h in range(HL):
                    if h == 3 and tb + 2 < NB:
                        issue_x(tb + 2)   # prefetch next block's x mid-B
                    # B0 heads are all-diagonal and short: place one filler
                    # group right before the exp-latency-exposed tail
                    emit_b_head(tb, h, mid_filler=pop_filler if tb == 0 else None)
                    for _ in range(2):
                        if c_pend:
                            emit_c_group()
                    for _ in range(1 if tb == 0 else 2):
                        pop_filler()
                while fstate[0] < len(filler):
                    pop_filler()
            c_pend.extend((tb, m2) for m2 in range(16))

        # drain the last block's output projection; B work is done, so
        # rotate over the mm banks too (5-deep pipelining instead of 2)
        with nc.named_scope("C3"):
            alt = 0
            while c_pend:
                if alt % 5 < 3:
                    emit_c_group(ptag="mm", pbufs=3)
                else:
                    emit_c_group()
                alt += 1
    nc.compile()
    return nc


# de-interleave permutation for rope channels: x1 (even) -> 0:32, x2 (odd) -> 32:64
_PERM = np.concatenate([np.arange(0, ROPE, 2), np.arange(1, ROPE, 2), np.arange(ROPE, hD)])


def prepare_inputs(x, cos, sin, Wq, Wkv, Wo, q_gain):
    """Host-side sharding + layout prep. Returns list of 8 in_maps."""
    x = np.asarray(x, np.float32)
    cos = np.asarray(cos, np.float32)
    sin = np.asarray(sin, np.float32)
    Wq = np.asarray(Wq, np.float32)
    Wkv = np.asarray(Wkv, np.float32)
    Wo = np.asarray(Wo, np.float32)
    q_gain = np.asarray(q_gain, np.float32)
    bf = ml_dtypes.bfloat16

    # rope tables in de-interleaved order: C = [cos; cos], S = [-sin; +sin]
    cosb = np.ascontiguousarray(np.concatenate([cos.T, cos.T], axis=0)).astype(bf)
    sinb = np.ascontiguousarray(np.concatenate([-sin.T, sin.T], axis=0)).astype(bf)

    # additive causal masks for diagonal s-tiles, r = j - 4*ib
    p = np.arange(128)[:, None]
    f = np.arange(512)[None, :]
    maskb = np.zeros((128, 4, 512), np.float32)
    for r in range(4):
        maskb[:, r, :] = np.where(p + 128 * r > f, -1e9, 0.0)
    maskb = maskb.astype(bf)

    onesb = np.ones((128, 128), np.float32)
    scale = 1.0 / np.sqrt(hD)

    # x: per batch [128, 16, T] partition-major contiguous bf16
    xb = [np.ascontiguousarray(
        x[b].T.reshape(16, 128, T).transpose(1, 0, 2)).astype(bf) for b in range(B)]

    in_maps = []
    for c in range(8):
        b, hf = divmod(c, 2)
        heads = np.arange(hf * HL, (hf + 1) * HL)
        Wq_h = Wq.reshape(H, hD, D)[heads] * (q_gain[heads, None, None] * scale)
        Wq_h = Wq_h[:, _PERM, :]                                     # de-interleave rope chans
        kvh = np.arange(hf * GL, (hf + 1) * GL)
        Wkv_r = Wkv.reshape(Hkv, 2 * hD, D)[kvh]
        Wk_h = Wkv_r[:, :hD, :][:, _PERM, :]
        Wv_h = Wkv_r[:, hD:, :]
        Wo_h = Wo[:, hf * HL * hD:(hf + 1) * HL * hD]

        # [D, M] transposed weights -> [128, 16, M] partition-major contiguous
        WqT = Wq_h.reshape(HL * hD, D).T        # [D, 1024]
        wq_host = np.ascontiguousarray(
            WqT.reshape(16, 128, HL, hD).transpose(1, 2, 0, 3)).astype(bf)
        WkT = Wk_h.reshape(GL * hD, D).T        # [D, 256]
        wk_host = np.ascontiguousarray(
            WkT.reshape(16, 128, GL * hD).transpose(1, 0, 2)).astype(bf)
        WvT = Wv_h.reshape(GL * hD, D).T
        wv_host = np.ascontiguousarray(
            WvT.reshape(16, 128, GL * hD).transpose(1, 0, 2)).astype(bf)

        in_maps.append({
            "xb": xb[b],
            "wq": wq_host,
            "wk": wk_host,
            "wv": wv_host,
            "wo": np.ascontiguousarray(Wo_h.T).astype(bf),
            "cosb": cosb, "sinb": sinb, "maskb": maskb,
            "onesb": onesb.astype(bf),
        })
    return in_maps


_NC_CACHE = {}


def kernel(x, cos, sin, Wq, Wkv, Wo, q_gain, _trace=False):
    if "nc" not in _NC_CACHE:
        _NC_CACHE["nc"] = build_nc()
    nc = _NC_CACHE["nc"]
    in_maps = prepare_inputs(x, cos, sin, Wq, Wkv, Wo, q_gain)
    res = run_bass_kernel_spmd(nc, in_maps, core_ids=list(range(8)), trace=_trace)
    if _trace:
        _NC_CACHE["last_results"] = res
    out = np.empty((B, T, D), np.float32)
    for b in range(B):
        acc = res.results[2 * b]["outT"] + res.results[2 * b + 1]["outT"]
        out[b] = acc.T
    return out


# revision 42
# speedup vs baseline: 1.0190x; 1.0148x over previous
"""Causal self-attention (GQA + partial RoPE + q_gain) Trainium2 Bass kernel.

Model: B=4, T=2048, D=2048, H=16 q-heads, Hkv=4 kv-heads, hD=128, ROPE=64.
Sharding: 8 cores = 4 batches x 2 head-halves (heads hf*8..hf*8+7, kv heads 2hf, 2hf+1).
Wq/Wkv column-sharded, Wo row-sharded; host sums the two partial outputs per batch.

v10 (~486us, from v2 baseline 660us):
  - host pre-transposes x/Wq/Wk/Wv into partition-major contiguous bf16 blocks
    (v2's strided f32r weight DMAs took 8-20us each and delayed the first
    matmul to t=33us). All matmuls bf16xbf16 (walrus rejects f32r x bf16).
  - steady-state DMA queues: sync = x tiles + output stores; gpsimd = wqh,
    rope swaps, consts, wo. scalar queue only carries wkg/wvg during A0 when
    the ACT engine is idle (engine-issued DMAs block the engine).
  - A(tb+1) projection groups are interleaved into B(tb) as PE filler
    (B0 had 13us of exp-latency bubbles with no C work to hide them); during
    B0 they sit on the pc PSUM banks, which are free until C(0) exists.
    x for block tb+2 prefetched from the middle of B(tb).
  - diagonal score tiles compute only the causally-valid columns
    (scores/exp/pdn/AV restricted to [128r:512]); pt is bf16 so the
    narrow moving operands still run at 1 cycle/row.
  - V projection packs two 256-wide token-chunks into one full PSUM bank
    (second half start=False self-initializes on the pending-zero bytes):
    a half-written bank would leave pending-zero bytes that silently
    discard the DVE-written causal mask of a later diagonal score tile.
  - softmax denominator: non-diagonal exp tiles pre-summed on DVE (bf16 2x)
    in pairs/quads, then one ones-stationary pass per group instead of one
    PE pass per tile (the v2 scheme cost 79us of pure PE overhead).
  - stile triple-buffered so the final C3 drain pipelines.
"""
import numpy as np
import ml_dtypes

import concourse.bass as bass
import concourse.tile as tile
from concourse import bacc, mybir
from concourse.bass_utils import run_bass_kernel_spmd
from contextlib import ExitStack

F32 = mybir.dt.float32
F32R = mybir.dt.float32r
BF16 = mybir.dt.bfloat16
AF = mybir.ActivationFunctionType
AOp = mybir.AluOpType

B, T, D = 4, 2048, 2048
H, Hkv = 16, 4
hD = 128
ROPE = 64
NB = T // 512          # 4 blocks of 512 tokens
HL = H // 2            # 8 heads per core
GL = Hkv // 2          # 2 kv heads per core


def build_nc():
    nc = bacc.Bacc(trn_type="TRN2", target_bir_lowering=False, debug=False)
    xb = nc.dram_tensor("xb", [128, 16, T], BF16, kind="ExternalInput").ap()
    wq = nc.dram_tensor("wq", [128, HL, 16, hD], BF16, kind="ExternalInput").ap()
    wk = nc.dram_tensor("wk", [128, 16, GL * hD], BF16, kind="ExternalInput").ap()
    wv = nc.dram_tensor("wv", [128, 16, GL * hD], BF16, kind="ExternalInput").ap()
    wo = nc.dram_tensor("wo", [HL * hD, D], BF16, kind="ExternalInput").ap()
    cosb = nc.dram_tensor("cosb", [ROPE, T], BF16, kind="ExternalInput").ap()
    sinb = nc.dram_tensor("sinb", [ROPE, T], BF16, kind="ExternalInput").ap()
    maskb = nc.dram_tensor("maskb", [128, 4, 512], BF16, kind="ExternalInput").ap()
    onesb = nc.dram_tensor("onesb", [128, 128], BF16, kind="ExternalInput").ap()
    outT = nc.dram_tensor("outT", [D, T], F32, kind="ExternalOutput").ap()

    with tile.TileContext(nc) as tc, ExitStack() as ctx:
        const = ctx.enter_context(tc.tile_pool(name="const", bufs=1))
        wpool = ctx.enter_context(tc.tile_pool(name="wpool", bufs=1))
        persist = ctx.enter_context(tc.tile_pool(name="persist", bufs=1))
        xpool = ctx.enter_context(tc.tile_pool(name="xp", bufs=48))
        qbp = ctx.enter_context(tc.tile_pool(name="qbp", bufs=18))
        swpool = ctx.enter_context(tc.tile_pool(name="swp", bufs=2))
        wqp = ctx.enter_context(tc.tile_pool(name="wqp", bufs=3))
        rtmp = ctx.enter_context(tc.tile_pool(name="rtmp", bufs=1))
        ptp = ctx.enter_context(tc.tile_pool(name="ptp", bufs=8))
        pt2p = ctx.enter_context(tc.tile_pool(name="pt2p", bufs=4))
        rpsp = ctx.enter_context(tc.tile_pool(name="rpsp", bufs=1))
        otp = ctx.enter_context(tc.tile_pool(name="otp", bufs=2))
        stilep = ctx.enter_context(tc.tile_pool(name="stilep", bufs=3))
        ps = ctx.enter_context(tc.tile_pool(name="ps", bufs=1, space="PSUM"))

        # rope tables first on gpsimd (needed by A0's K rope); tones/tmask
        # deferred (first needed in B0) so A0's wqh loads go out earlier
        tcos = const.tile([ROPE, T], BF16, tag="tcos")
        nc.gpsimd.dma_start(tcos[:], cosb)
        tsin = const.tile([ROPE, T], BF16, tag="tsin")
        nc.gpsimd.dma_start(tsin[:], sinb)
        tones = const.tile([128, 128], BF16, tag="tones")
        tmask = const.tile([128, 4, 512], BF16, tag="tmask")

        # K/V weights on the scalar queue: ACT is idle during A0.
        # wkg in 4 chunks so the first K matmul starts after ~1/4 transfer.
        wkg = wpool.tile([128, 16, GL * hD], BF16, tag="wkg")
        for q in range(4):
            nc.scalar.dma_start(wkg[:, 4 * q:4 * (q + 1), :], wk[:, 4 * q:4 * (q + 1), :])
        wvg = wpool.tile([128, 16, GL * hD], BF16, tag="wvg")
        wo_t = []
        for j in range(HL):
            w = wpool.tile([128, D], BF16, tag=f"wo{j}", name=f"wo{j}")
            wo_t.append(w)

        KT = persist.tile([128, GL, T], BF16, tag="KT")
        VT = persist.tile([128, T // 128, GL * hD], BF16, tag="VT")

        qbs = {tb: [] for tb in range(NB)}   # tb -> 8 qb tiles
        xts = {}                             # tb -> 16 x tiles
        OTh = {}                             # tb -> OT tile [128, HL, 512] bf16
        c_pend = []                          # queued phase-C groups: (tb, m2)

        def issue_x(tb):
            tsl = slice(512 * tb, 512 * (tb + 1))
            lst = []
            for d in range(16):
                xt = xpool.tile([128, 512], BF16, tag="xt", name=f"xt{tb}_{d}")
                nc.sync.dma_start(xt[:], xb[:, d, tsl])
                lst.append(xt)
            xts[tb] = lst

        def xap(tb, d):
            return xts[tb][d][:]

        def emit_c_group(ptag="pc", pbufs=2):
            tbc, m2 = c_pend.pop(0)
            msl = slice(128 * m2, 128 * (m2 + 1))
            csl = slice(512 * tbc, 512 * (tbc + 1))
            pc = ps.tile([128, 512], F32, tag=ptag, bufs=pbufs, name=f"pc{tbc}_{m2}")
            for j in range(HL):
                nc.tensor.matmul(pc[:], wo_t[j][:, msl], OTh[tbc][:][:, j, :],
                                 start=(j == 0), stop=(j == HL - 1))
            stile = stilep.tile([128, 512], F32, tag="stile", name=f"st{tbc}_{m2}")
            nc.vector.tensor_copy(stile[:], pc[:])
            nc.sync.dma_start(outT[msl, csl], stile[:])

        def rope(dst, src_ap, tsl, nm):
            # dst[0:64, :] = src[0:64]*C + swap(src)*S (C/S hold the +-sin trick)
            sw = swpool.tile([ROPE, 512], BF16, tag="sw", name=f"sw{nm}")
            nc.gpsimd.dma_start(sw[0:32, :], src_ap[32:64])
            nc.gpsimd.dma_start(sw[32:64, :], src_ap[0:32])
            ts_ = rtmp.tile([ROPE, 512], F32R, tag="ts_", name=f"ts{nm}")
            tc_ = rtmp.tile([ROPE, 512], F32R, tag="tc_", name=f"tc{nm}")
            nc.vector.tensor_mul(ts_[:], sw[:], tsin[:, tsl])
            nc.vector.tensor_mul(tc_[:], src_ap[0:ROPE], tcos[:, tsl])
            nc.vector.tensor_tensor(out=dst[0:ROPE], in0=tc_[:], in1=ts_[:], op=AOp.add)

        def make_a_groups(tb, ptag="mm", pbufs=3):
            """Closures for A(tb)'s matmul groups: 2 K, 2 V-pair, 8 Q."""
            tsl = slice(512 * tb, 512 * (tb + 1))
            groups = []
            wqhs = {}

            def load_wqh(h):
                # sync queue: gpsimd is in-order behind rope-swap DMAs that
                # themselves wait on DVE, which stalled Q groups on wqh
                if h < HL and h not in wqhs:
                    wqh = wqp.tile([128, 16, hD], BF16, tag="wqh", name=f"wq{tb}_{h}")
                    nc.sync.dma_start(wqh[:], wq[:, h])
                    wqhs[h] = wqh

            def kg(g):
                pk = ps.tile([128, 512], F32, tag=ptag, bufs=pbufs, name=f"pk{tb}_{g}")
                for d in range(16):
                    nc.tensor.matmul(pk[:], wkg[:][:, d, 128 * g:128 * (g + 1)],
                                     xap(tb, d), start=(d == 0), stop=(d == 15))
                nc.vector.tensor_copy(KT[:][:, g, tsl], pk[:])
                rope(KT[:][:, g, tsl], KT[:][:, g, tsl], tsl, f"k{tb}_{g}")

            def vg(tp):
                # two token-chunks packed into one full PSUM bank: the second
                # half's d=0 matmul uses start=False and lands on the
                # pending-zero bytes left by the first half's start=True, so
                # it self-initializes. Leaving the bank half-written would
                # poison later diagonal score tiles (mask DVE-write discarded
                # on pending-zero bytes).
                pv = ps.tile([128, 512], F32, tag=ptag, bufs=pbufs, name=f"pv{tb}_{tp}")
                for half in range(2):
                    tloc = 2 * tp + half
                    for d in range(16):
                        nc.tensor.matmul(pv[:, 256 * half:256 * (half + 1)],
                                         xts[tb][d][:, 128 * tloc:128 * (tloc + 1)],
                                         wvg[:][:, d, :],
                                         start=(d == 0 and half == 0),
                                         stop=(d == 15 and half == 1),
                                         skip_group_check=(half == 1))
                nc.vector.tensor_copy(
                    VT[:][:, 4 * tb + 2 * tp:4 * tb + 2 * tp + 2, :], pv[:])

            def qg(h):
                load_wqh(h)          # no-op if already prefetched
                load_wqh(h + 1)      # prefetch next heads' weights
                load_wqh(h + 2)
                wqh = wqhs.pop(h)
                pq = ps.tile([128, 512], F32, tag=ptag, bufs=pbufs, name=f"pq{tb}_{h}")
                for d in range(16):
                    nc.tensor.matmul(pq[:], wqh[:][:, d, :],
                                     xap(tb, d), start=(d == 0), stop=(d == 15))
                qb = qbp.tile([128, 512], BF16, tag="qb", name=f"qb{tb}_{h}")
                nc.vector.tensor_copy(qb[:], pq[:])
                rope(qb[:], qb[:], tsl, f"q{tb}_{h}")
                qbs[tb].append(qb)

            load_wqh(0)
            load_wqh(1)
            for g in range(GL):
                groups.append(lambda g=g: kg(g))
            for tp in range(2):
                groups.append(lambda tp=tp: vg(tp))
            for h in range(HL):
                groups.append(lambda h=h: qg(h))
            return groups

        def emit_b_head(tb, h, mid_filler=None):
            nj = 4 * tb + 4
            ndiag = 4 * tb          # j < ndiag are full-width (non-diagonal)
            g = h // (HL // GL)
            po = ps.tile([128, 512], F32, tag="po", bufs=2, name=f"po{tb}_{h}")
            pdn = ps.tile([128, 512], F32, tag="pdn", bufs=1, name=f"pd{tb}_{h}")
            pts = []
            slices = []
            pairs = []
            started = [False]

            def pdn_mm(src_ap, sl, last):
                st_, started[0] = not started[0], True
                nc.tensor.matmul(pdn[:, sl], tones[:], src_ap,
                                 start=st_, stop=last,
                                 skip_group_check=not st_)

            def emit_av(j):
                sl = slices[j]
                part = sl != slice(0, 512)
                if j >= ndiag:
                    # diagonal: per-j denominator on the valid columns
                    pdn_mm(pts[j][:][:, sl], sl, j == nj - 1)
                elif j & 1:
                    # non-diagonal pair (j-1, j) folded into a running DVE
                    # bf16 sum; ONE denominator pass per head for all
                    # non-diagonal tiles (values <= ndiag, fine in bf16)
                    p2 = pt2p.tile([128, 512], BF16, tag="pt2",
                                   name=f"p2{tb}_{h}_{j}")
                    nc.vector.tensor_tensor(out=p2[:], in0=pts[j - 1][:],
                                            in1=pts[j][:], op=AOp.add)
                    if pairs:
                        run = pt2p.tile([128, 512], BF16, tag="pt2",
                                        name=f"run{tb}_{h}_{j}")
                        nc.vector.tensor_tensor(out=run[:], in0=pairs.pop()[:],
                                                in1=p2[:], op=AOp.add)
                    else:
                        run = p2
                    if j == ndiag - 1:
                        pdn_mm(run[:], slice(0, 512), False)
                    else:
                        pairs.append(run)
                nc.tensor.matmul(po[:, sl], VT[:][:, j, 128 * g:128 * (g + 1)],
                                 pts[j][:][:, sl], start=(j == 0), stop=(j == nj - 1),
                                 skip_group_check=part)

            for j in range(nj):
                if j == nj - 1 and mid_filler is not None:
                    # PE filler just before the exp-latency-exposed tail
                    mid_filler()
                r = j - 4 * tb
                st = ps.tile([128, 512], F32, tag="mm", bufs=3,
                             name=f"s{tb}_{h}_{j}")
                if r >= 0:
                    sl = slice(128 * r, 512)
                    # mask-init on ACT (same table set as Exp, no reload):
                    # DVE is the congested engine during B phases
                    nc.scalar.activation(st[:, sl], tmask[:][:, r, sl], AF.Copy)
                    nc.tensor.matmul(st[:, sl], KT[:][:, g, 128 * j:128 * (j + 1)],
                                     qbs[tb][h][:][:, sl], start=False, stop=True,
                                     skip_group_check=True)
                else:
                    sl = slice(0, 512)
                    nc.tensor.matmul(st[:], KT[:][:, g, 128 * j:128 * (j + 1)],
                                     qbs[tb][h][:], start=True, stop=True)
                pt = ptp.tile([128, 512], BF16, tag="pt", name=f"p{tb}_{h}_{j}")
                nc.scalar.activation(pt[:, sl], st[:, sl], AF.Exp)
                pts.append(pt)
                slices.append(sl)
                if j >= 3:
                    emit_av(j - 3)
            for j in range(max(nj - 3, 0), nj):
                emit_av(j)
            rps = rpsp.tile([128, 512], F32, tag="rps", name=f"r{tb}_{h}")
            nc.vector.reciprocal_approx_fast(rps[:], pdn[:])
            nc.vector.tensor_tensor(out=OTh[tb][:][:, h, :], in0=po[:],
                                    in1=rps[:], op=AOp.mult)

        # ---------------- A0 standalone ----------------
        with nc.named_scope("A0"):
            issue_x(0)
            nc.scalar.dma_start(wvg[:], wv)
            for grp in make_a_groups(0):
                grp()
            issue_x(1)   # A1 filler groups inside B0 need these early
            # deferred consts (first needed in B0), then Wo (needed in B1)
            nc.gpsimd.dma_start(tones[:], onesb)
            nc.gpsimd.dma_start(tmask[:], maskb)
            for j in range(HL):
                nc.gpsimd.dma_start(wo_t[j][:], wo[128 * j:128 * (j + 1), :])

        # ---------------- B(tb) with A(tb+1) + C(tb-1) filler ----------------
        for tb in range(NB):
            with nc.named_scope(f"B{tb}"):
                OTh[tb] = otp.tile([128, HL, 512], BF16, tag="OT", name=f"OT{tb}")
                filler = []
                if tb + 1 < NB:
                    # during B0 the pc banks are free (no C work queued yet):
                    # keep A1 filler groups off the score-tile rotation
                    filler = (make_a_groups(tb + 1, ptag="pc", pbufs=2)
                              if tb == 0 else make_a_groups(tb + 1))
                fstate = [0]

                def pop_filler():
                    if fstate[0] < len(filler):
                        filler[fstate[0]]()
                        fstate[0] += 1

                for h in range(HL):
                    if h == 3 and tb + 2 < NB:
                        issue_x(tb + 2)   # prefetch next block's x mid-B
                    # B0 heads are all-diagonal and short: place one filler
                    # group right before the exp-latency-exposed tail
                    emit_b_head(tb, h, mid_filler=pop_filler if tb == 0 else None)
                    for _ in range(2):
                        if c_pend:
                            emit_c_group()
                    for _ in range(1 if tb == 0 else 2):
                        pop_filler()
                while fstate[0] < len(filler):
                    pop_filler()
            c_pend.extend((tb, m2) for m2 in range(16))

        # drain the last block's output projection; B work is done, so
        # rotate over the mm banks too (5-deep pipelining instead of 2)
        with nc.named_scope("C3"):
            alt = 0
            while c_pend:
                if alt % 5 < 3:
                    emit_c_group(ptag="mm", pbufs=3)
                else:
                    emit_c_group()
                alt += 1
    nc.compile()
    return nc


# de-interleave permutation for rope channels: x1 (even) -> 0:32, x2 (odd) -> 32:64
_PERM = np.concatenate([np.arange(0, ROPE, 2), np.arange(1, ROPE, 2), np.arange(ROPE, hD)])


def prepare_inputs(x, cos, sin, Wq, Wkv, Wo, q_gain):
    """Host-side sharding + layout prep. Returns list of 8 in_maps."""
    x = np.asarray(x, np.float32)
    cos = np.asarray(cos, np.float32)
    sin = np.asarray(sin, np.float32)
    Wq = np.asarray(Wq, np.float32)
    Wkv = np.asarray(Wkv, np.float32)
    Wo = np.asarray(Wo, np.float32)
    q_gain = np.asarray(q_gain, np.float32)
    bf = ml_dtypes.bfloat16

    # rope tables in de-interleaved order: C = [cos; cos], S = [-sin; +sin]
    cosb = np.ascontiguousarray(np.concatenate([cos.T, cos.T], axis=0)).astype(bf)
    sinb = np.ascontiguousarray(np.concatenate([-sin.T, sin.T], axis=0)).astype(bf)

    # additive causal masks for diagonal s-tiles, r = j - 4*ib
    p = np.arange(128)[:, None]
    f = np.arange(512)[None, :]
    maskb = np.zeros((128, 4, 512), np.float32)
    for r in range(4):
        maskb[:, r, :] = np.where(p + 128 * r > f, -1e9, 0.0)
    maskb = maskb.astype(bf)

    onesb = np.ones((128, 128), np.float32)
    scale = 1.0 / np.sqrt(hD)

    # x: per batch [128, 16, T] partition-major contiguous bf16
    xb = [np.ascontiguousarray(
        x[b].T.reshape(16, 128, T).transpose(1, 0, 2)).astype(bf) for b in range(B)]

    in_maps = []
    for c in range(8):
        b, hf = divmod(c, 2)
        heads = np.arange(hf * HL, (hf + 1) * HL)
        Wq_h = Wq.reshape(H, hD, D)[heads] * (q_gain[heads, None, None] * scale)
        Wq_h = Wq_h[:, _PERM, :]                                     # de-interleave rope chans
        kvh = np.arange(hf * GL, (hf + 1) * GL)
        Wkv_r = Wkv.reshape(Hkv, 2 * hD, D)[kvh]
        Wk_h = Wkv_r[:, :hD, :][:, _PERM, :]
        Wv_h = Wkv_r[:, hD:, :]
        Wo_h = Wo[:, hf * HL * hD:(hf + 1) * HL * hD]

        # [D, M] transposed weights -> [128, 16, M] partition-major contiguous
        WqT = Wq_h.reshape(HL * hD, D).T        # [D, 1024]
        wq_host = np.ascontiguousarray(
            WqT.reshape(16, 128, HL, hD).transpose(1, 2, 0, 3)).astype(bf)
        WkT = Wk_h.reshape(GL * hD, D).T        # [D, 256]
        wk_host = np.ascontiguousarray(
            WkT.reshape(16, 128, GL * hD).transpose(1, 0, 2)).astype(bf)
        WvT = Wv_h.reshape(GL * hD, D).T
        wv_host = np.ascontiguousarray(
            WvT.reshape(16, 128, GL * hD).transpose(1, 0, 2)).astype(bf)

        in_maps.append({
            "xb": xb[b],
            "wq": wq_host,
            "wk": wk_host,
            "wv": wv_host,
            "wo": np.ascontiguousarray(Wo_h.T).astype(bf),
            "cosb": cosb, "sinb": sinb, "maskb": maskb,
            "onesb": onesb.astype(bf),
        })
    return in_maps


_NC_CACHE = {}


def kernel(x, cos, sin, Wq, Wkv, Wo, q_gain, _trace=False):
    if "nc" not in _NC_CACHE:
        _NC_CACHE["nc"] = build_nc()
    nc = _NC_CACHE["nc"]
    in_maps = prepare_inputs(x, cos, sin, Wq, Wkv, Wo, q_gain)
    res = run_bass_kernel_spmd(nc, in_maps, core_ids=list(range(8)), trace=_trace)
    if _trace:
        _NC_CACHE["last_results"] = res
    out = np.empty((B, T, D), np.float32)
    for b in range(B):
        acc = res.results[2 * b]["outT"] + res.results[2 * b + 1]["outT"]
        out[b] = acc.T
    return out


# revision 43
# speedup vs baseline: 1.0268x; 1.0077x over previous
"""Causal self-attention (GQA + partial RoPE + q_gain) Trainium2 Bass kernel.

Model: B=4, T=2048, D=2048, H=16 q-heads, Hkv=4 kv-heads, hD=128, ROPE=64.
Sharding: 8 cores = 4 batches x 2 head-halves (heads hf*8..hf*8+7, kv heads 2hf, 2hf+1).
Wq/Wkv column-sharded, Wo row-sharded; host sums the two partial outputs per batch.

v10 (~486us, from v2 baseline 660us):
  - host pre-transposes x/Wq/Wk/Wv into partition-major contiguous bf16 blocks
    (v2's strided f32r weight DMAs took 8-20us each and delayed the first
    matmul to t=33us). All matmuls bf16xbf16 (walrus rejects f32r x bf16).
  - steady-state DMA queues: sync = x tiles + output stores; gpsimd = wqh,
    rope swaps, consts, wo. scalar queue only carries wkg/wvg during A0 when
    the ACT engine is idle (engine-issued DMAs block the engine).
  - A(tb+1) projection groups are interleaved into B(tb) as PE filler
    (B0 had 13us of exp-latency bubbles with no C work to hide them); during
    B0 they sit on the pc PSUM banks, which are free until C(0) exists.
    x for block tb+2 prefetched from the middle of B(tb).
  - diagonal score tiles compute only the causally-valid columns
    (scores/exp/pdn/AV restricted to [128r:512]); pt is bf16 so the
    narrow moving operands still run at 1 cycle/row.
  - V projection packs two 256-wide token-chunks into one full PSUM bank
    (second half start=False self-initializes on the pending-zero bytes):
    a half-written bank would leave pending-zero bytes that silently
    discard the DVE-written causal mask of a later diagonal score tile.
  - softmax denominator: non-diagonal exp tiles pre-summed on DVE (bf16 2x)
    in pairs/quads, then one ones-stationary pass per group instead of one
    PE pass per tile (the v2 scheme cost 79us of pure PE overhead).
  - stile triple-buffered so the final C3 drain pipelines.
"""
import numpy as np
import ml_dtypes

import concourse.bass as bass
import concourse.tile as tile
from concourse import bacc, mybir
from concourse.bass_utils import run_bass_kernel_spmd
from contextlib import ExitStack

F32 = mybir.dt.float32
F32R = mybir.dt.float32r
BF16 = mybir.dt.bfloat16
AF = mybir.ActivationFunctionType
AOp = mybir.AluOpType

B, T, D = 4, 2048, 2048
H, Hkv = 16, 4
hD = 128
ROPE = 64
NB = T // 512          # 4 blocks of 512 tokens
HL = H // 2            # 8 heads per core
GL = Hkv // 2          # 2 kv heads per core


def build_nc():
    nc = bacc.Bacc(trn_type="TRN2", target_bir_lowering=False, debug=False)
    xb = nc.dram_tensor("xb", [128, 16, T], BF16, kind="ExternalInput").ap()
    wq = nc.dram_tensor("wq", [128, HL, 16, hD], BF16, kind="ExternalInput").ap()
    wk = nc.dram_tensor("wk", [128, 16, GL * hD], BF16, kind="ExternalInput").ap()
    wv = nc.dram_tensor("wv", [128, 16, GL * hD], BF16, kind="ExternalInput").ap()
    wo = nc.dram_tensor("wo", [HL * hD, D], BF16, kind="ExternalInput").ap()
    cosb = nc.dram_tensor("cosb", [ROPE, T], BF16, kind="ExternalInput").ap()
    sinb = nc.dram_tensor("sinb", [ROPE, T], BF16, kind="ExternalInput").ap()
    maskb = nc.dram_tensor("maskb", [128, 4, 512], BF16, kind="ExternalInput").ap()
    onesb = nc.dram_tensor("onesb", [128, 128], BF16, kind="ExternalInput").ap()
    outT = nc.dram_tensor("outT", [D, T], F32, kind="ExternalOutput").ap()

    with tile.TileContext(nc) as tc, ExitStack() as ctx:
        const = ctx.enter_context(tc.tile_pool(name="const", bufs=1))
        wpool = ctx.enter_context(tc.tile_pool(name="wpool", bufs=1))
        persist = ctx.enter_context(tc.tile_pool(name="persist", bufs=1))
        xpool = ctx.enter_context(tc.tile_pool(name="xp", bufs=48))
        qbp = ctx.enter_context(tc.tile_pool(name="qbp", bufs=18))
        swpool = ctx.enter_context(tc.tile_pool(name="swp", bufs=2))
        wqp = ctx.enter_context(tc.tile_pool(name="wqp", bufs=3))
        rtmp = ctx.enter_context(tc.tile_pool(name="rtmp", bufs=1))
        ptp = ctx.enter_context(tc.tile_pool(name="ptp", bufs=8))
        pt2p = ctx.enter_context(tc.tile_pool(name="pt2p", bufs=4))
        rpsp = ctx.enter_context(tc.tile_pool(name="rpsp", bufs=1))
        otp = ctx.enter_context(tc.tile_pool(name="otp", bufs=2))
        stilep = ctx.enter_context(tc.tile_pool(name="stilep", bufs=3))
        ps = ctx.enter_context(tc.tile_pool(name="ps", bufs=1, space="PSUM"))

        # rope tables first on gpsimd (needed by A0's K rope); tones/tmask
        # deferred (first needed in B0) so A0's wqh loads go out earlier
        tcos = const.tile([ROPE, T], BF16, tag="tcos")
        nc.gpsimd.dma_start(tcos[:], cosb)
        tsin = const.tile([ROPE, T], BF16, tag="tsin")
        nc.gpsimd.dma_start(tsin[:], sinb)
        tones = const.tile([128, 128], BF16, tag="tones")
        tmask = const.tile([128, 4, 512], BF16, tag="tmask")

        # K/V weights on the scalar queue: ACT is idle during A0.
        # wkg in 4 chunks so the first K matmul starts after ~1/4 transfer.
        wkg = wpool.tile([128, 16, GL * hD], BF16, tag="wkg")
        for q in range(4):
            nc.scalar.dma_start(wkg[:, 4 * q:4 * (q + 1), :], wk[:, 4 * q:4 * (q + 1), :])
        wvg = wpool.tile([128, 16, GL * hD], BF16, tag="wvg")
        wo_t = []
        for j in range(HL):
            w = wpool.tile([128, D], BF16, tag=f"wo{j}", name=f"wo{j}")
            wo_t.append(w)

        KT = persist.tile([128, GL, T], BF16, tag="KT")
        VT = persist.tile([128, T // 128, GL * hD], BF16, tag="VT")

        qbs = {tb: [] for tb in range(NB)}   # tb -> 8 qb tiles
        xts = {}                             # tb -> 16 x tiles
        OTh = {}                             # tb -> OT tile [128, HL, 512] bf16
        c_pend = []                          # queued phase-C groups: (tb, m2)

        def issue_x(tb):
            tsl = slice(512 * tb, 512 * (tb + 1))
            lst = []
            for d in range(16):
                xt = xpool.tile([128, 512], BF16, tag="xt", name=f"xt{tb}_{d}")
                nc.sync.dma_start(xt[:], xb[:, d, tsl])
                lst.append(xt)
            xts[tb] = lst

        def xap(tb, d):
            return xts[tb][d][:]

        def emit_c_group(ptag="pc", pbufs=2):
            tbc, m2 = c_pend.pop(0)
            msl = slice(128 * m2, 128 * (m2 + 1))
            csl = slice(512 * tbc, 512 * (tbc + 1))
            pc = ps.tile([128, 512], F32, tag=ptag, bufs=pbufs, name=f"pc{tbc}_{m2}")
            for j in range(HL):
                nc.tensor.matmul(pc[:], wo_t[j][:, msl], OTh[tbc][:][:, j, :],
                                 start=(j == 0), stop=(j == HL - 1))
            stile = stilep.tile([128, 512], F32, tag="stile", name=f"st{tbc}_{m2}")
            nc.vector.tensor_copy(stile[:], pc[:])
            nc.sync.dma_start(outT[msl, csl], stile[:])

        def rope(dst, src_ap, tsl, nm):
            # dst[0:64, :] = src[0:64]*C + swap(src)*S (C/S hold the +-sin trick)
            sw = swpool.tile([ROPE, 512], BF16, tag="sw", name=f"sw{nm}")
            nc.gpsimd.dma_start(sw[0:32, :], src_ap[32:64])
            nc.gpsimd.dma_start(sw[32:64, :], src_ap[0:32])
            ts_ = rtmp.tile([ROPE, 512], F32R, tag="ts_", name=f"ts{nm}")
            tc_ = rtmp.tile([ROPE, 512], F32R, tag="tc_", name=f"tc{nm}")
            nc.vector.tensor_mul(ts_[:], sw[:], tsin[:, tsl])
            nc.vector.tensor_mul(tc_[:], src_ap[0:ROPE], tcos[:, tsl])
            nc.vector.tensor_tensor(out=dst[0:ROPE], in0=tc_[:], in1=ts_[:], op=AOp.add)

        def make_a_groups(tb, ptag="mm", pbufs=3):
            """Closures for A(tb)'s matmul groups: 2 K, 2 V-pair, 8 Q."""
            tsl = slice(512 * tb, 512 * (tb + 1))
            groups = []
            wqhs = {}

            def load_wqh(h):
                # sync queue: gpsimd is in-order behind rope-swap DMAs that
                # themselves wait on DVE, which stalled Q groups on wqh
                if h < HL and h not in wqhs:
                    wqh = wqp.tile([128, 16, hD], BF16, tag="wqh", name=f"wq{tb}_{h}")
                    nc.sync.dma_start(wqh[:], wq[:, h])
                    wqhs[h] = wqh

            def kg(g):
                pk = ps.tile([128, 512], F32, tag=ptag, bufs=pbufs, name=f"pk{tb}_{g}")
                for d in range(16):
                    nc.tensor.matmul(pk[:], wkg[:][:, d, 128 * g:128 * (g + 1)],
                                     xap(tb, d), start=(d == 0), stop=(d == 15))
                nc.vector.tensor_copy(KT[:][:, g, tsl], pk[:])
                rope(KT[:][:, g, tsl], KT[:][:, g, tsl], tsl, f"k{tb}_{g}")

            def vg(tp):
                # two token-chunks packed into one full PSUM bank: the second
                # half's d=0 matmul uses start=False and lands on the
                # pending-zero bytes left by the first half's start=True, so
                # it self-initializes. Leaving the bank half-written would
                # poison later diagonal score tiles (mask DVE-write discarded
                # on pending-zero bytes).
                pv = ps.tile([128, 512], F32, tag=ptag, bufs=pbufs, name=f"pv{tb}_{tp}")
                for half in range(2):
                    tloc = 2 * tp + half
                    for d in range(16):
                        nc.tensor.matmul(pv[:, 256 * half:256 * (half + 1)],
                                         xts[tb][d][:, 128 * tloc:128 * (tloc + 1)],
                                         wvg[:][:, d, :],
                                         start=(d == 0 and half == 0),
                                         stop=(d == 15 and half == 1),
                                         skip_group_check=(half == 1))
                nc.vector.tensor_copy(
                    VT[:][:, 4 * tb + 2 * tp:4 * tb + 2 * tp + 2, :], pv[:])

            def qg(h):
                load_wqh(h)          # no-op if already prefetched
                load_wqh(h + 1)      # prefetch next heads' weights
                load_wqh(h + 2)
                wqh = wqhs.pop(h)
                pq = ps.tile([128, 512], F32, tag=ptag, bufs=pbufs, name=f"pq{tb}_{h}")
                for d in range(16):
                    nc.tensor.matmul(pq[:], wqh[:][:, d, :],
                                     xap(tb, d), start=(d == 0), stop=(d == 15))
                qb = qbp.tile([128, 512], BF16, tag="qb", name=f"qb{tb}_{h}")
                nc.vector.tensor_copy(qb[:], pq[:])
                rope(qb[:], qb[:], tsl, f"q{tb}_{h}")
                qbs[tb].append(qb)

            load_wqh(0)
            load_wqh(1)
            for g in range(GL):
                groups.append(lambda g=g: kg(g))
            for tp in range(2):
                groups.append(lambda tp=tp: vg(tp))
            for h in range(HL):
                groups.append(lambda h=h: qg(h))
            return groups

        def emit_b_head(tb, h, mid_filler=None):
            nj = 4 * tb + 4
            ndiag = 4 * tb          # j < ndiag are full-width (non-diagonal)
            g = h // (HL // GL)
            po = ps.tile([128, 512], F32, tag="po", bufs=2, name=f"po{tb}_{h}")
            pdn = ps.tile([128, 512], F32, tag="pdn", bufs=1, name=f"pd{tb}_{h}")
            pts = []
            slices = []
            pairs = []
            started = [False]

            def pdn_mm(src_ap, sl, last):
                st_, started[0] = not started[0], True
                nc.tensor.matmul(pdn[:, sl], tones[:], src_ap,
                                 start=st_, stop=last,
                                 skip_group_check=not st_)

            def emit_av(j):
                sl = slices[j]
                part = sl != slice(0, 512)
                if j >= ndiag:
                    # diagonal: per-j denominator on the valid columns
                    pdn_mm(pts[j][:][:, sl], sl, j == nj - 1)
                elif j & 1:
                    # non-diagonal pair (j-1, j): one DVE bf16 add, then a
                    # single denominator pass per pair (or per quad for the
                    # deeper blocks, where DVE still has slack)
                    p2 = pt2p.tile([128, 512], BF16, tag="pt2",
                                   name=f"p2{tb}_{h}_{j}")
                    nc.vector.tensor_tensor(out=p2[:], in0=pts[j - 1][:],
                                            in1=pts[j][:], op=AOp.add)
                    if ndiag >= 8 and (j & 3) == 1 and j + 2 < ndiag:
                        pairs.append(p2)     # defer: quad partner comes at j+2
                    elif ndiag >= 8 and (j & 3) == 3 and pairs:
                        p4 = pt2p.tile([128, 512], BF16, tag="pt2",
                                       name=f"p4{tb}_{h}_{j}")
                        nc.vector.tensor_tensor(out=p4[:], in0=pairs.pop()[:],
                                                in1=p2[:], op=AOp.add)
                        pdn_mm(p4[:], slice(0, 512), False)
                    else:
                        pdn_mm(p2[:], slice(0, 512), False)
                nc.tensor.matmul(po[:, sl], VT[:][:, j, 128 * g:128 * (g + 1)],
                                 pts[j][:][:, sl], start=(j == 0), stop=(j == nj - 1),
                                 skip_group_check=part)

            for j in range(nj):
                if j == nj - 1 and mid_filler is not None:
                    # PE filler just before the exp-latency-exposed tail
                    mid_filler()
                r = j - 4 * tb
                st = ps.tile([128, 512], F32, tag="mm", bufs=3,
                             name=f"s{tb}_{h}_{j}")
                if r >= 0:
                    sl = slice(128 * r, 512)
                    # mask-init on ACT (same table set as Exp, no reload):
                    # DVE is the congested engine during B phases
                    nc.scalar.activation(st[:, sl], tmask[:][:, r, sl], AF.Copy)
                    nc.tensor.matmul(st[:, sl], KT[:][:, g, 128 * j:128 * (j + 1)],
                                     qbs[tb][h][:][:, sl], start=False, stop=True,
                                     skip_group_check=True)
                else:
                    sl = slice(0, 512)
                    nc.tensor.matmul(st[:], KT[:][:, g, 128 * j:128 * (j + 1)],
                                     qbs[tb][h][:], start=True, stop=True)
                pt = ptp.tile([128, 512], BF16, tag="pt", name=f"p{tb}_{h}_{j}")
                nc.scalar.activation(pt[:, sl], st[:, sl], AF.Exp)
                pts.append(pt)
                slices.append(sl)
                if j >= 3:
                    emit_av(j - 3)
            for j in range(max(nj - 3, 0), nj):
                emit_av(j)
            rps = rpsp.tile([128, 512], F32, tag="rps", name=f"r{tb}_{h}")
            nc.vector.reciprocal_approx_fast(rps[:], pdn[:])
            nc.vector.tensor_tensor(out=OTh[tb][:][:, h, :], in0=po[:],
                                    in1=rps[:], op=AOp.mult)

        # ---------------- A0 standalone ----------------
        with nc.named_scope("A0"):
            issue_x(0)
            nc.scalar.dma_start(wvg[:], wv)
            for grp in make_a_groups(0):
                grp()
            issue_x(1)   # A1 filler groups inside B0 need these early
            # deferred consts (first needed in B0), then Wo (needed in B1)
            nc.gpsimd.dma_start(tones[:], onesb)
            nc.gpsimd.dma_start(tmask[:], maskb)
            for j in range(HL):
                nc.gpsimd.dma_start(wo_t[j][:], wo[128 * j:128 * (j + 1), :])

        # ---------------- B(tb) with A(tb+1) + C(tb-1) filler ----------------
        for tb in range(NB):
            with nc.named_scope(f"B{tb}"):
                OTh[tb] = otp.tile([128, HL, 512], BF16, tag="OT", name=f"OT{tb}")
                filler = []
                if tb + 1 < NB:
                    # during B0 the pc banks are free (no C work queued yet):
                    # keep A1 filler groups off the score-tile rotation
                    filler = (make_a_groups(tb + 1, ptag="pc", pbufs=2)
                              if tb == 0 else make_a_groups(tb + 1))
                fstate = [0]

                def pop_filler():
                    if fstate[0] < len(filler):
                        filler[fstate[0]]()
                        fstate[0] += 1

                for h in range(HL):
                    if h == 3 and tb + 2 < NB:
                        issue_x(tb + 2)   # prefetch next block's x mid-B
                    # B0 heads are all-diagonal and short: place one filler
                    # group right before the exp-latency-exposed tail
                    emit_b_head(tb, h, mid_filler=pop_filler if tb == 0 else None)
                    for _ in range(2):
                        if c_pend:
                            emit_c_group()
                    for _ in range(1 if tb == 0 else 2):
                        pop_filler()
                while fstate[0] < len(filler):
                    pop_filler()
            c_pend.extend((tb, m2) for m2 in range(16))

        # drain the last block's output projection; B work is done, so
        # rotate over the mm banks too (5-deep pipelining instead of 2)
        with nc.named_scope("C3"):
            alt = 0
            while c_pend:
                if alt % 5 < 3:
                    emit_c_group(ptag="mm", pbufs=3)
                else:
                    emit_c_group()
                alt += 1
    nc.compile()
    return nc


# de-interleave permutation for rope channels: x1 (even) -> 0:32, x2 (odd) -> 32:64
_PERM = np.concatenate([np.arange(0, ROPE, 2), np.arange(1, ROPE, 2), np.arange(ROPE, hD)])


def prepare_inputs(x, cos, sin, Wq, Wkv, Wo, q_gain):
    """Host-side sharding + layout prep. Returns list of 8 in_maps."""
    x = np.asarray(x, np.float32)
    cos = np.asarray(cos, np.float32)
    sin = np.asarray(sin, np.float32)
    Wq = np.asarray(Wq, np.float32)
    Wkv = np.asarray(Wkv, np.float32)
    Wo = np.asarray(Wo, np.float32)
    q_gain = np.asarray(q_gain, np.float32)
    bf = ml_dtypes.bfloat16

    # rope tables in de-interleaved order: C = [cos; cos], S = [-sin; +sin]
    cosb = np.ascontiguousarray(np.concatenate([cos.T, cos.T], axis=0)).astype(bf)
    sinb = np.ascontiguousarray(np.concatenate([-sin.T, sin.T], axis=0)).astype(bf)

    # additive causal masks for diagonal s-tiles, r = j - 4*ib
    p = np.arange(128)[:, None]
    f = np.arange(512)[None, :]
    maskb = np.zeros((128, 4, 512), np.float32)
    for r in range(4):
        maskb[:, r, :] = np.where(p + 128 * r > f, -1e9, 0.0)
    maskb = maskb.astype(bf)

    onesb = np.ones((128, 128), np.float32)
    scale = 1.0 / np.sqrt(hD)

    # x: per batch [128, 16, T] partition-major contiguous bf16
    xb = [np.ascontiguousarray(
        x[b].T.reshape(16, 128, T).transpose(1, 0, 2)).astype(bf) for b in range(B)]

    in_maps = []
    for c in range(8):
        b, hf = divmod(c, 2)
        heads = np.arange(hf * HL, (hf + 1) * HL)
        Wq_h = Wq.reshape(H, hD, D)[heads] * (q_gain[heads, None, None] * scale)
        Wq_h = Wq_h[:, _PERM, :]                                     # de-interleave rope chans
        kvh = np.arange(hf * GL, (hf + 1) * GL)
        Wkv_r = Wkv.reshape(Hkv, 2 * hD, D)[kvh]
        Wk_h = Wkv_r[:, :hD, :][:, _PERM, :]
        Wv_h = Wkv_r[:, hD:, :]
        Wo_h = Wo[:, hf * HL * hD:(hf + 1) * HL * hD]

        # [D, M] transposed weights -> [128, 16, M] partition-major contiguous
        WqT = Wq_h.reshape(HL * hD, D).T        # [D, 1024]
        wq_host = np.ascontiguousarray(
            WqT.reshape(16, 128, HL, hD).transpose(1, 2, 0, 3)).astype(bf)
        WkT = Wk_h.reshape(GL * hD, D).T        # [D, 256]
        wk_host = np.ascontiguousarray(
            WkT.reshape(16, 128, GL * hD).transpose(1, 0, 2)).astype(bf)
        WvT = Wv_h.reshape(GL * hD, D).T
        wv_host = np.ascontiguousarray(
            WvT.reshape(16, 128, GL * hD).transpose(1, 0, 2)).astype(bf)

        in_maps.append({
            "xb": xb[b],
            "wq": wq_host,
            "wk": wk_host,
            "wv": wv_host,
            "wo": np.ascontiguousarray(Wo_h.T).astype(bf),
            "cosb": cosb, "sinb": sinb, "maskb": maskb,
            "onesb": onesb.astype(bf),
        })
    return in_maps


_NC_CACHE = {}


def kernel(x, cos, sin, Wq, Wkv, Wo, q_gain, _trace=False):
    if "nc" not in _NC_CACHE:
        _NC_CACHE["nc"] = build_nc()
    nc = _NC_CACHE["nc"]
    in_maps = prepare_inputs(x, cos, sin, Wq, Wkv, Wo, q_gain)
    res = run_bass_kernel_spmd(nc, in_maps, core_ids=list(range(8)), trace=_trace)
    if _trace:
        _NC_CACHE["last_results"] = res
    out = np.empty((B, T, D), np.float32)
    for b in range(B):
        acc = res.results[2 * b]["outT"] + res.results[2 * b + 1]["outT"]
        out[b] = acc.T
    return out
